# revision 30
# baseline (speedup 1.0000x reference)
"""HCNN (known-U) recurrence kernel for 8 Trainium2 NeuronCores.

Model (see reference): 80 sequential steps of
    state' = tanh(cat(post_state, u)) @ A            A: (2112, 2048) fp32
with teacher forcing post_state[:, :128] = y during the 64 past steps,
outputs = 64 past errors then 16 forecasts (first 128 state components).

Strategy
--------
Data-parallel over batch: 256 = 8 cores x 32. Each core runs the full
recurrence for its batch slice; no collectives.

Per-core per-step matmul x @ A with batch M=32 would waste 3/4 of the
128-wide PE array, so the A columns are split into 4 interleaved groups
and computed by 4 concurrent column-tiled matmuls (tile_position=(0,32j))
sharing the array. Data is fp16 (single pass): the teacher-forced
recurrence is strongly contractive, emulation shows end-to-end output
error ~1.5e-4 relative vs the fp32 reference.

Column interleave: state column s lives in col-group j=(s//32)%4 at free
offset 32*(s//128) + s%32. With that mapping the psum holding state'
(batch on partitions within each 32-group) turns into the next step's
stationary operand layout via DVE 32x32 block-transposes: block (j, m')
lands at partitions [32j:32j+32] of k-tile m' -- exactly where matmul
round m' reads it.

Pipelining (see _build_program): each k-round is split into lo/hi psum
halves in two different psum BANKS; the hi half of a pair reuses the lo
half's stationary (redundant LDWEIGHTS deleted post-schedule -- the LDW
port is the binding resource at N=256). Six tail ks run lo-only so the
lo bank closes ~1us early, their hi halves are deferred past the next
step's dependency-free y/u rounds, and the tanh+transpose chain for
both halves overlaps the surrounding matmuls. Tile's per-instruction
PE semaphore increments are stripped to the referenced thresholds
(~10k -> ~230) -- the EVT_SEM write unit otherwise saturates and delays
dependent engines by ~1us. Step 0 runs on the host (init_state is a
broadcast row). Measured: 337.8us vs 445.9us for the unpipelined
version; PE busy 94.4%, steady step 3.78us vs 3.64us stream floor.
"""

import sys

for _p in ("/opt/trn_rl_repo", "/root/.axon_site/_ro/trn_rl_repo"):
    if _p not in sys.path:
        sys.path.insert(0, _p)

import numpy as np

N_STATE = 2048
N_U = 64
N_Y = 128
PAST = 64
FORE = 16
BATCH = 256
T = PAST + FORE          # 80 total steps; only 79 matmul steps needed
NSTEP = T - 1            # step t computes state_{t+1}; state_80 is unused
NDEV = NSTEP - 1         # step 0 runs on the host (init_state is a broadcast
                         # row, so s_1 is a cheap rank-192 GEMM + one row)
NK = 17                  # contraction tiles: 16 x 128 state + 1 x (64 u + 64 pad)
KDIM = NK * 128          # 2176 padded contraction size
N_CORES = 8
B = BATCH // N_CORES     # 32 per core


def _build_program():
    import concourse.bass as bass
    import concourse.tile as tile
    from concourse import mybir

    F32 = mybir.dt.float32
    F16 = mybir.dt.float16

    nc = bass.Bass("TRN2", target_bir_lowering=False, debug=False,
                   num_devices=N_CORES)

    A_ext = nc.declare_dram_parameter("A_re", [KDIM, 2048], F16, isOutput=False)
    ytanhT_ext = nc.declare_dram_parameter("ytanhT", [128, PAST * B], F16, isOutput=False)
    utanhT_ext = nc.declare_dram_parameter("utanhT", [128, NSTEP * B], F16, isOutput=False)
    ywrap_ext = nc.declare_dram_parameter("ywrap", [128, (PAST - 2) * B], F32, isOutput=False)
    initxT_ext = nc.declare_dram_parameter("initxT", [128, 512], F16, isOutput=False)
    out_ext = nc.declare_dram_parameter("outbuf", [128, NDEV * B], F32, isOutput=True)

    with tile.TileContext(nc) as tc:
        with tc.tile_pool(name="const", bufs=1) as cpool, \
             tc.tile_pool(name="xbuf", bufs=2) as xpool, \
             tc.tile_pool(name="th", bufs=2) as thpool, \
             tc.tile_pool(name="psum", bufs=2, space="PSUM") as pspool:

            # DMA order = first-use order: init x strips + y (feed step 0's
            # first rounds), then A tiles in round emission order so step 0
            # can begin as soon as the first tiles land. ywrap is only read
            # by the DVE output path, so it loads last.
            A_sb = cpool.tile([128, NK * 2048], F16, tag="A")
            ytanhT = cpool.tile([128, PAST * B], F16, tag="yt")
            utanhT = cpool.tile([128, NSTEP * B], F16, tag="ut")
            ywrap = cpool.tile([128, (PAST - 2) * B], F32, tag="yw")
            outbuf = cpool.tile([128, NDEV * B], F32, tag="ob")
            xlo = xpool.tile([128, 256], F16, tag="xlo")
            xhi = xpool.tile([128, 256], F16, tag="xhi")

            # critical first-use loads on the sync ring; the bulk A tiles
            # split across both HWDGE rings (sync + scalar) so issue and
            # transfer bandwidth add up.
            # Only the first ~16 steps' y/u columns are needed before the A
            # load finishes; the rest stream in afterwards, freeing ~0.9MB
            # of pre-A HBM bandwidth (~3us earlier steady state).
            YUC = 17 * B
            nc.sync.dma_start(out=ytanhT[:, 0:YUC], in_=ytanhT_ext[:, 0:YUC])
            nc.scalar.dma_start(out=utanhT[:, 0:YUC],
                                in_=utanhT_ext[:, 0:YUC])
            nc.sync.dma_start(out=A_sb[:, 0:2048], in_=A_ext[0:128, :])
            nc.scalar.dma_start(out=A_sb[:, 2048 * 16:2048 * 17],
                                in_=A_ext[128 * 16:128 * 17, :])
            nc.sync.dma_start(out=xlo[:], in_=initxT_ext[:, 0:256])
            nc.sync.dma_start(out=xhi[:], in_=initxT_ext[:, 256:512])
            korder = [0, 16] + list(range(1, 16))
            for i, k in enumerate(range(1, 16)):
                eng = nc.sync if i % 2 == 0 else nc.scalar
                eng.dma_start(out=A_sb[:, 2048 * k:2048 * (k + 1)],
                              in_=A_ext[128 * k:128 * (k + 1), :])
            nc.sync.dma_start(out=ytanhT[:, YUC:], in_=ytanhT_ext[:, YUC:])
            nc.scalar.dma_start(out=utanhT[:, YUC:], in_=utanhT_ext[:, YUC:])
            nc.scalar.dma_start(out=ywrap[:], in_=ywrap_ext[:])

            def lhs_for(t, k, lo, hi):
                if k == 0:
                    if t < PAST:
                        return ytanhT[:, B * t:B * (t + 1)]
                    return lo[:, 0:32]
                if k == 16:
                    return utanhT[:, B * t:B * (t + 1)]
                if k < 8:
                    return lo[:, 32 * k:32 * (k + 1)]
                return hi[:, 32 * (k - 8):32 * (k - 7)]

            # Pair ks: lo+hi halves back-to-back per position so the hi MM
            # reuses the lo MM's stationary (its redundant LDWEIGHTS is
            # deleted by _dedup_ldweights — the LDW port, 4x~33ns per round,
            # is the binding resource at N=256). Tail ks: lo-only rounds
            # first so the lo psum bank closes ~0.9us before the step ends,
            # giving the tanh+transpose chain a head start; their hi halves
            # (re-loading the same weights) close the step.
            pair_ks = [0, 16] + list(range(1, 10))
            tail_ks = list(range(10, 16))

            def mm(t, xl, xh, ps, k, j, half, start, stop):
                c0 = 0 if half == 0 else 256
                nc.tensor.matmul(
                    ps[32 * j:32 * (j + 1), 0:256],
                    lhs_for(t, k, xl, xh),
                    A_sb[:, 2048 * k + 512 * j + c0:
                         2048 * k + 512 * j + c0 + 256],
                    start=start, stop=stop,
                    tile_position=(0, 32 * j),
                )

            def emit_tail_hi(pend):
                # deferred hi-tail matmuls of step pt, then (only now, so
                # the writers precede the reader in emission order and Tile
                # derives reader-after-writer deps) the hi tanh+transpose
                # chain producing x_{pt+1}'s hi strips.
                pt, pxl, pxh, pps_hi, pth_hi, pnhi = pend
                for idx, k in enumerate(tail_ks):
                    for j in range(4):
                        mm(pt, pxl, pxh, pps_hi, k, j, 1,
                           False, idx == len(tail_ks) - 1)
                for c0 in (0, 128):
                    nc.scalar.activation(pth_hi[:, c0:c0 + 128],
                                         pps_hi[:, c0:c0 + 128],
                                         mybir.ActivationFunctionType.Tanh)
                    nc.vector.transpose(pnhi[:, c0:c0 + 128],
                                        pth_hi[:, c0:c0 + 128])

            warm_state = {}

            def warm_rounds(n, xl, xh):
                if "t" not in warm_state:
                    warm_state["t"] = pspool.tile([128, 512], F32,
                                                  tag="warm", bufs=1,
                                                  name="wsc")
                wsc = warm_state["t"]
                wlhs = lhs_for(1, 0, xl, xh)
                for _ in range(n):
                    for j in range(4):
                        nc.tensor.matmul(
                            wsc[32 * j:32 * (j + 1), :], wlhs,
                            A_sb[:, 512 * j:512 * (j + 1)],
                            start=True, stop=True,
                            tile_position=(0, 32 * j),
                        )

            # Step t's hi-tail rounds are deferred into iteration t+1,
            # emitted after t+1's dependency-free y/u pair rounds: they fill
            # the PE while step t's tanh+transpose chain produces x_{t+1},
            # so the first state round (k=1) of t+1 sees its operand ready.
            pending = None
            for t in range(1, NSTEP):
                last = t == NSTEP - 1
                # lo = psum cols 0:256 (state cols 0:1024), hi = 256:512.
                # Separate psum tiles padded to a full 2KB bank: ACT may only
                # read one bank while the PE writes another (same-bank
                # PE-write + ACT-read is fatal and Tile serializes it).
                # The final step only needs psum cols 0:32 (the forecast).
                ps_lo = pspool.tile([128, 512], F32, tag="pslo")
                if last:
                    if pending is not None:
                        emit_tail_hi(pending)
                        pending = None
                    for idx, k in enumerate(korder):
                        lhsT = lhs_for(t, k, xlo, xhi)
                        for j in range(4):
                            nc.tensor.matmul(
                                ps_lo[32 * j:32 * (j + 1), 0:32],
                                lhsT,
                                A_sb[:, 2048 * k + 512 * j:
                                     2048 * k + 512 * j + 32],
                                start=idx == 0, stop=idx == len(korder) - 1,
                                tile_position=(0, 32 * j),
                            )
                else:
                    ps_hi = pspool.tile([128, 512], F32, tag="pshi")
                    # hoistable head: y/u rounds (teacher-forced y only
                    # exists for past steps; the forecast k=0 round reads
                    # the recurrent state and would head-of-line-block).
                    head = [0, 16] if t < PAST else [16]
                    for idx, k in enumerate(head):
                        for j in range(4):
                            mm(t, xlo, xhi, ps_lo, k, j, 0, idx == 0, False)
                            mm(t, xlo, xhi, ps_hi, k, j, 1, idx == 0, False)
                        if t == 1 and k == 0:
                            # Warm-up inside the A-load window: discarded
                            # matmuls (weights deduped, sem incs stripped)
                            # flip the HAM clock gate to 8/8 early and keep
                            # every PE-idle DMA gap under the ~3.4us
                            # re-throttle window, so real steps run at
                            # 2.4GHz even when HBM is slow. The first burst
                            # sits between the k0 and k16 head pairs — it
                            # depends only on ytanhT+A_k0, already resident,
                            # while k16's operands are still in flight on
                            # the scalar ring.
                            warm_rounds(10, xlo, xhi)
                    if pending is not None:
                        emit_tail_hi(pending)
                        pending = None
                    for k in (k for k in pair_ks if k not in head):
                        for j in range(4):
                            mm(t, xlo, xhi, ps_lo, k, j, 0, False, False)
                            mm(t, xlo, xhi, ps_hi, k, j, 1, False, False)
                        if t == 1:
                            warm_rounds(4, xlo, xhi)
                    for idx, k in enumerate(tail_ks):
                        for j in range(4):
                            mm(t, xlo, xhi, ps_lo, k, j, 0,
                               False, idx == len(tail_ks) - 1)
                        if t == 1 and idx < len(tail_ks) - 1:
                            warm_rounds(4, xlo, xhi)

                    th_lo = thpool.tile([128, 256], F16, tag="thlo")
                    nlo = xpool.tile([128, 256], F16, tag="xlo")
                    th_hi = thpool.tile([128, 256], F16, tag="thhi")
                    nhi = xpool.tile([128, 256], F16, tag="xhi")
                    pending = (t, xlo, xhi, ps_hi, th_hi, nhi)
                    for c0 in (0, 128):
                        nc.scalar.activation(th_lo[:, c0:c0 + 128],
                                             ps_lo[:, c0:c0 + 128],
                                             mybir.ActivationFunctionType.Tanh)
                        nc.vector.transpose(nlo[:, c0:c0 + 128],
                                            th_lo[:, c0:c0 + 128])

                # output slot s = t-1 (expectation = cols 0:128 of state',
                # living in ps_lo[:, 0:32] across all partition groups);
                # emitted after the transposes so the DVE unblocks them first.
                s = t - 1
                if t + 1 < PAST:
                    nc.vector.tensor_sub(outbuf[:, B * s:B * (s + 1)],
                                         ps_lo[:, 0:32],
                                         ywrap[:, B * s:B * (s + 1)])
                else:
                    nc.vector.tensor_copy(outbuf[:, B * s:B * (s + 1)],
                                          ps_lo[:, 0:32])

                if not last:
                    xlo, xhi = nlo, nhi

                # stream finished output slots out while compute continues;
                # the extra chunk at s=75 keeps the post-loop DMA (and its
                # slow HBM-write completion semaphore) down to two slots.
                if s % 8 == 7:
                    nc.sync.dma_start(out=out_ext[:, B * (s - 7):B * (s + 1)],
                                      in_=outbuf[:, B * (s - 7):B * (s + 1)])
                elif s == 75:
                    nc.sync.dma_start(out=out_ext[:, B * 72:B * 76],
                                      in_=outbuf[:, B * 72:B * 76])

            nc.sync.dma_start(out=out_ext[:, B * 76:],
                              in_=outbuf[:, B * 76:])

    _dedup_ldweights(nc)
    _strip_dead_pe_incs(nc)
    _split_multi_waits(nc)
    return nc


def _strip_dead_pe_incs(nc):
    """Tile gives every PE instruction a counting-sem increment; with ~10k
    matmuls the EVT_SEM write unit (~26ns per inc) saturates and dependent
    engines observe psum completion ~0.5-1.0us late. Keep only increments
    whose cumulative value some wait references, and remap thresholds to
    ranks within the kept set."""
    sem = None
    # discover the PE engine counting sem name (unique per build id)
    for f in nc.m.functions:
        for b in f.blocks:
            for ins in b.instructions:
                if type(ins).__name__ != 'InstMatmult':
                    continue
                si = ins.sync_info
                for u in (si.on_update or []) if si else []:
                    if u.update_mode == 'sem-inc':
                        sem = u.ant_name
                        break
                if sem:
                    break
            if sem:
                break
        if sem:
            break
    if sem is None:
        return 0

    # collect updates (program order across blocks) and referenced values
    upds = []      # (inst, update-obj) in order
    refs = set()
    for f in nc.m.functions:
        for b in f.blocks:
            for ins in b.instructions:
                si = ins.sync_info
                if si is None:
                    continue
                for u in (si.on_update or []):
                    if u.ant_name == sem:
                        if u.update_mode != 'sem-inc' or u.update_value != 1:
                            return 0  # unexpected pattern; abort
                        upds.append((ins, u))
                for w in (si.on_wait or []):
                    if w.ant_name == sem:
                        if w.wait_mode != 'sem-ge-imm' or w.wait_value is None:
                            return 0
                        refs.add(w.wait_value)

    n = len(upds)
    keep = sorted(v for v in refs if 1 <= v <= n)
    keep_set = set(keep)
    # new threshold for wait value v = number of kept incs with index <= v
    import bisect
    from concourse import mybir as _mb

    stripped = 0
    idx_base = 0
    for f in nc.m.functions:
        for b in f.blocks:
            for ins in b.instructions:
                si = ins.sync_info
                if si is None:
                    continue
                changed = False
                new_upd = []
                for u in (si.on_update or []):
                    if u.ant_name == sem:
                        idx_base += 1
                        if idx_base not in keep_set:
                            stripped += 1
                            changed = True
                            continue
                    new_upd.append(u)
                new_wait = []
                for w in (si.on_wait or []):
                    if w.ant_name == sem:
                        w = _mb.SyncWait(
                            sync_type=w.sync_type, id=w.id,
                            ant_name=w.ant_name, wait_mode=w.wait_mode,
                            wait_value=bisect.bisect_right(keep, w.wait_value),
                            wait_reg=w.wait_reg)
                        changed = True
                    new_wait.append(w)
                if changed:
                    ins.sync_info = _mb.SyncInfo(on_wait=new_wait,
                                                 on_update=new_upd)
    return stripped


def _dedup_ldweights(nc):
    """Tile lowers each matmul into InstLdweights + InstMatmult. Our lo/hi
    psum-half pairs reload an identical stationary at the same PE tile
    position; the LDW port (one column per cycle, serialized across the four
    col positions) is the binding resource at N=256, so drop the redundant
    loads. Only loads with no sync obligations are removed."""
    removed = 0
    for f in nc.m.functions:
        for b in f.blocks:
            last = {}
            out = []
            changed = False
            for ins in b.instructions:
                tn = type(ins).__name__
                if tn == 'InstLdweights':
                    w = ins.ins[0]
                    pos = ins.tile_position
                    key = pos[1] if pos else None
                    sig = (w.memref, w.offset, str(w.ap), str(w.dtype), pos)
                    si = ins.sync_info
                    clean = si is None or (not si.on_wait and not si.on_update)
                    if last.get(key) == sig and clean:
                        removed += 1
                        changed = True
                        continue
                    last[key] = sig
                out.append(ins)
            if changed:
                b.instructions = out
    return removed


def _split_multi_waits(nc):
    """This walrus build accepts at most one sem wait per instruction; Tile
    sometimes emits more. Hoist extras onto nops inserted just before the
    instruction in the same engine stream."""
    from concourse import mybir

    n = 0
    for f in nc.m.functions:
        for b in f.blocks:
            insts = b.instructions
            out = []
            changed = False
            for ins in insts:
                si = ins.sync_info
                if si is not None and len(si.on_wait) > 1:
                    waits = list(si.on_wait)
                    for w in waits[:-1]:
                        n += 1
                        out.append(mybir.InstNoOp(
                            name=f"I-waitsplit-{n}",
                            engine=ins.engine,
                            ins=[], outs=[],
                            bass_nofuse=True,
                            sync_info=mybir.SyncInfo(on_wait=[w], on_update=[]),
                        ))
                    ins.sync_info = mybir.SyncInfo(
                        on_wait=[waits[-1]], on_update=list(si.on_update))
                    changed = True
                out.append(ins)
            if changed:
                b.instructions = out


def _host_inputs(U, Y, A, init_state):
    """Build the per-core input maps (all pre-tanh / pre-transpose work) and
    the host-computed step-0 outputs. state_0 is a broadcast of init_state,
    so s_1 = tanh(y_0)@A_y + tanh(u_0)@A_u + (one broadcast row) is cheap
    host math; the device runs steps 1..78 from x_1."""
    A = np.asarray(A, np.float32)
    U = np.asarray(U, np.float32)
    Y = np.asarray(Y, np.float32)
    init_state = np.asarray(init_state, np.float32)

    A_pad = np.zeros((KDIM, N_STATE), np.float16)
    A_pad[:N_STATE + N_U] = A.astype(np.float16)
    # column interleave: col s -> (j=(s//32)%4, free 32*(s//128)+s%32)
    A_re = np.ascontiguousarray(
        A_pad.reshape(KDIM, 16, 4, 32).transpose(0, 2, 1, 3).reshape(KDIM, 2048))

    # host step 0
    init_tanh = np.tanh(init_state[0])                             # (2048,)
    s1 = (np.tanh(Y[0]) @ A[:N_Y]
          + np.tanh(U[0]) @ A[N_STATE:]
          + init_tanh[N_Y:N_STATE] @ A[N_Y:N_STATE])               # (256, 2048)
    err1 = s1[:, :N_Y] - Y[1]
    post1 = s1.copy()
    post1[:, :N_Y] = Y[1]
    tpost1 = np.tanh(post1).astype(np.float16)                     # (256, 2048)

    ytanh = np.tanh(Y).astype(np.float16)                          # (64, 256, 128)
    utanh = np.tanh(U[:NSTEP]).astype(np.float16)                  # (79, 256, 64)

    in_maps = []
    for c in range(N_CORES):
        b0 = c * B
        yt = np.ascontiguousarray(
            ytanh[:, b0:b0 + B, :].transpose(0, 2, 1)              # (64, 128, 32)
            .transpose(1, 0, 2).reshape(128, PAST * B))
        ut = np.zeros((128, NSTEP * B), np.float16)
        ut[:N_U] = (utanh[:, b0:b0 + B, :].transpose(0, 2, 1)      # (79, 64, 32)
                    .transpose(1, 0, 2).reshape(N_U, NSTEP * B))
        # ywrap slot s (device step t=s+1) holds Y[s+2]
        yw = (Y[2:PAST, b0:b0 + B, :].reshape(PAST - 2, B, 4, 32)
              .transpose(0, 2, 1, 3)                               # (62, 4, 32b, 32cc)
              .reshape(PAST - 2, 128, 32)
              .transpose(1, 0, 2).reshape(128, (PAST - 2) * B))
        # x_1 strips: initxT[kk, 32k + b] = tanh(post_1)[b0+b, 128k + kk]
        x1T = np.ascontiguousarray(
            tpost1[b0:b0 + B].reshape(B, 16, 128).transpose(2, 1, 0)
            .reshape(128, 512))
        in_maps.append({
            "A_re": A_re,
            "ytanhT": yt,
            "utanhT": np.ascontiguousarray(ut),
            "ywrap": np.ascontiguousarray(yw.astype(np.float32)),
            "initxT": x1T,
        })
    aux = {
        "err0": init_state[0, :N_Y][None, :] - Y[0],
        "err1": err1,
    }
    return in_maps, aux


def _assemble(obufs, aux):
    out = np.empty((T, BATCH, N_Y), np.float32)
    out[0] = aux["err0"]
    out[1] = aux["err1"]
    for c in range(N_CORES):
        # [32j+b, 32s+cc] = out[s+2, c*B+b, 32j+cc]
        ob4 = obufs[c].reshape(4, 32, NDEV, 32)                    # (j, b, s, cc)
        out[2:, c * B:(c + 1) * B, :] = (
            ob4.transpose(2, 1, 0, 3).reshape(NDEV, B, N_Y))
    return out


def kernel(U, Y, A, init_state):
    from concourse.bass_utils import run_bass_kernel_spmd

    nc = _build_program()
    in_maps, aux = _host_inputs(U, Y, A, init_state)
    res = run_bass_kernel_spmd(nc, in_maps, list(range(N_CORES)))
    return _assemble([res.results[c]["outbuf"] for c in range(N_CORES)], aux)


if __name__ == "__main__":
    rng = np.random.default_rng(0)
    U = rng.standard_normal((T, BATCH, N_U)).astype(np.float32)
    Y = rng.standard_normal((PAST, BATCH, N_Y)).astype(np.float32)
    A = (rng.standard_normal((N_STATE + N_U, N_STATE)) * 0.02).astype(np.float32)
    init = rng.standard_normal((1, N_STATE)).astype(np.float32)
    o = kernel(U=U, Y=Y, A=A, init_state=init)
    print("kernel out:", o.shape, o.dtype)

